# revision 1
# baseline (speedup 1.0000x reference)
"""Trainium2 Bass kernel for nn_BulkSpaceGenerator.

Math: the fast-marching scan g_k = g_{k-1} + (1/(k+1))(c_k - g_{k-1}) starting
from c_0 yields the running mean g_k = mean(c_0..c_k); the mean over k of those
is sum_j w_j c_j with w_j = (1/K)(H_K - H_j) (harmonic numbers). Since
c_j = tokens @ W[:, j*D:(j+1)*D] + b[j*D:(j+1)*D], the whole module is

    out = tokens @ W_eff + b_eff,   W_eff = sum_j w_j W_j,  b_eff = sum_j w_j b_j

The kernel folds W -> W_eff on-device (DVE) and runs the (8192x1024)@(1024x1024)
matmul on the PE array, sharded over 8 cores as 4 feature-shards x 2
token-shards (minimizes per-core HBM traffic: W_slice + tokens/2 + out/8).

Layout per core (f in 0..3, t in 0..1, core = f*2 + t):
  tokT : (1024, 4096) f32  -- tokens^T slice, columns t*4096:(t+1)*4096
  wsl  : (1024, 2560) f32  -- W[:, j*1024 + f*256 : j*1024 + (f+1)*256], j-major
  bsl  : (256, 10)    f32  -- b[j*1024 + f*256 + d] transposed to (d, j)
  outT : (256, 4096)  f32  -- out^T slice (host reassembles full (4,2048,1024))
"""

import os
from contextlib import ExitStack

import numpy as np

import concourse.bass as bass
import concourse.tile as tile
from concourse import bacc, mybir
from concourse.bass_utils import run_bass_kernel_spmd

D_MODEL = 1024
BULK_DIM = 10
B, N = 4, 2048
BN = B * N                     # 8192 tokens
NCORES = 8
F_SHARDS = 4                   # feature shards (d dimension)
T_SHARDS = 2                   # token shards
DS = D_MODEL // F_SHARDS       # 256 output features per core
MS = BN // T_SHARDS            # 4096 tokens per core
KT = D_MODEL // 128            # 8 contraction k-tiles
DT = DS // 128                 # 2 output d-tiles of 128 per core
MCHUNK = 512                   # moving free dim per matmul
NMI = MS // MCHUNK             # 8 m-chunks per core

# w_j = (1/K) * (H_K - H_j), H_j = sum_{i=1..j} 1/i
_H = np.cumsum(1.0 / np.arange(1, BULK_DIM + 1))
W_COEF = ((_H[-1] - np.concatenate([[0.0], _H[:-1]])) / BULK_DIM).tolist()

# mode: "f32r" | "f32" | "bf16" keep f32 inputs on the wire; "f16" ships
# tokens and W as fp16 (half the load bytes, ~3.6e-4 rel err vs 1.5e-4 f32r)
MODE = os.environ.get("BULK_KERNEL_MODE", "f16")

_BUILD_CACHE = {}


def _build(mode: str) -> bass.Bass:
    f32 = mybir.dt.float32
    bf16 = mybir.dt.bfloat16
    wire_dt = mybir.dt.float16 if mode in ("f16",) else f32

    nc = bacc.Bacc("TRN2", target_bir_lowering=False, debug=False,
                   num_devices=NCORES)
    tokT = nc.dram_tensor("tokT", [D_MODEL, MS], wire_dt,
                          kind="ExternalInput").ap()
    wsl = nc.dram_tensor("wsl", [D_MODEL, BULK_DIM * DS], wire_dt,
                         kind="ExternalInput").ap()
    bsl = nc.dram_tensor("bsl", [DS, BULK_DIM], f32, kind="ExternalInput").ap()
    outT = nc.dram_tensor("outT", [DS, MS], f32, kind="ExternalOutput").ap()

    with tile.TileContext(nc) as tc, ExitStack() as ctx:
        wraw_pool = ctx.enter_context(
            tc.tile_pool(name="wraw",
                         bufs=KT if mode in ("f16",) else 2))
        weff_pool = ctx.enter_context(tc.tile_pool(name="weff", bufs=KT))
        tok_pool = ctx.enter_context(tc.tile_pool(name="tok", bufs=KT))
        bias_pool = ctx.enter_context(tc.tile_pool(name="bias", bufs=2 * DT))
        psum_pool = ctx.enter_context(
            tc.tile_pool(name="psum", bufs=8, space="PSUM"))
        out_pool = ctx.enter_context(tc.tile_pool(name="osb", bufs=4))
        weffc_pool = None
        if mode != "f32":
            weffc_pool = ctx.enter_context(tc.tile_pool(name="weffc", bufs=KT))

        mult = mybir.AluOpType.mult
        add = mybir.AluOpType.add
        mm_dtype = {"bf16": bf16, "f32r": mybir.dt.float32r, "f32": f32,
                    "f16": mybir.dt.float16}[mode]

        # zero operands for PE-warming no-op matmuls (memset can't write f32r
        # directly; produce via a rounding copy). zrhs is independent of any
        # input DMA so warm-up can start immediately.
        zf = bias_pool.tile([128, 512], f32, tag="zf")
        nc.vector.memset(zf[:], 0.0)
        # casts on ACT (idle early) so they don't delay the DVE fold chains
        # (ACT Copy with an f32r out dtype is unverified -> DVE for f32r)
        zcast = nc.scalar if mode == "f16" else nc.vector
        zmm = bias_pool.tile([128, 128], mm_dtype, tag="zmm")
        zcast.copy(zmm[:], zf[:, 0:128]) if mode == "f16" else \
            nc.vector.tensor_copy(zmm[:], zf[:, 0:128])
        zrhs = bias_pool.tile([128, 512], mm_dtype, tag="zrhs")
        zcast.copy(zrhs[:], zf[:]) if mode == "f16" else \
            nc.vector.tensor_copy(zrhs[:], zf[:])

        # ---- per k-tile: load W slice, fold W_eff, load tokens ----
        toks = []
        weffs = []
        for kt in range(KT):
            ksl = slice(kt * 128, (kt + 1) * 128)
            wr = wraw_pool.tile([128, BULK_DIM * DS], wire_dt)
            if mode in ("f16",):
                # split columns so the fold chain (j ascending) starts as
                # soon as the first half lands (subtile deps)
                hw = BULK_DIM * DS // 2
                nc.scalar.dma_start(wr[:, 0:hw], wsl[ksl, 0:hw])
                nc.scalar.dma_start(wr[:, hw:], wsl[ksl, hw:])
            else:
                nc.gpsimd.dma_start(wr[:], wsl[ksl, :])

            tk = tok_pool.tile([128, MS], mm_dtype)
            if mode in ("f16",):
                # no cast needed -> HWDGE queue, decoupled from W-load waits
                nc.sync.dma_start(tk[:], tokT[ksl, :])
            else:
                nc.gpsimd.dma_start(tk[:], tokT[ksl, :])  # SWDGE rounding cast
            toks.append(tk)

            fold = nc.vector
            we = weff_pool.tile([128, DS], f32)
            fold.tensor_scalar_mul(we[:], wr[:, 0:DS], W_COEF[0])
            for j in range(1, BULK_DIM - 1):
                fold.scalar_tensor_tensor(
                    we[:], wr[:, j * DS:(j + 1) * DS], W_COEF[j], we[:],
                    mult, add)
            j = BULK_DIM - 1
            if mode == "f32":
                fold.scalar_tensor_tensor(
                    we[:], wr[:, j * DS:(j + 1) * DS], W_COEF[j], we[:],
                    mult, add)
                weffs.append(we)
            else:
                # final fold step writes the matmul dtype directly (rounds)
                wc = weffc_pool.tile([128, DS], mm_dtype)
                fold.scalar_tensor_tensor(
                    wc[:], wr[:, j * DS:(j + 1) * DS], W_COEF[j], we[:],
                    mult, add)
                weffs.append(wc)

        # ---- fold bias: beff[d] = sum_j w_j b[j*D + d], per-partition ----
        biases = []
        bfold = nc.vector
        for dt_i in range(DT):
            bt = bias_pool.tile([128, BULK_DIM], f32, tag="bt")
            nc.sync.dma_start(bt[:], bsl[dt_i * 128:(dt_i + 1) * 128, :])
            be = bias_pool.tile([128, 1], f32, tag="be")
            bfold.tensor_scalar_mul(be[:], bt[:, 0:1], W_COEF[0])
            for j in range(1, BULK_DIM):
                bfold.scalar_tensor_tensor(
                    be[:], bt[:, j:j + 1], W_COEF[j], be[:], mult, add)
            biases.append(be)

        # ---- matmul: kt-outer in 2 halves (8 live psum banks each) ----
        # Between k-steps of the first half the PE is supply-gated on DMA;
        # zero-weight no-op matmuls keep its HAM clock at 8/8 (idle >3.4us
        # re-throttles the PE to 1.2 GHz).
        n_dummy = 4 if mode != "f32" else 0
        n_prewarm = 40 if mode != "f32" else 0
        half_mi = NMI // 2

        def evict(ps, dt_i, msl):
            ot = out_pool.tile([128, MCHUNK], f32, name="ot", tag="ot")
            if dt_i == 0:
                nc.scalar.add(ot[:], ps[:], biases[dt_i][:])
                nc.scalar.dma_start(
                    outT[dt_i * 128:(dt_i + 1) * 128, msl], ot[:])
            else:
                nc.vector.tensor_scalar_add(
                    ot[:], ps[:], biases[dt_i][:, 0:1])
                nc.sync.dma_start(
                    outT[dt_i * 128:(dt_i + 1) * 128, msl], ot[:])

        # half 0: kt-outer (matmuls chase the incoming DMA stream)
        psums = [[psum_pool.tile([128, MCHUNK], f32, name="ps", tag="ps")
                  for _ in range(DT)] for _ in range(half_mi)]
        # warm the PE's HAM clock before the first real matmul; these touch
        # only memset tiles, so they run during the load phase
        for _ in range(n_prewarm):
            nc.tensor.matmul(psums[0][0][:], lhsT=zmm[:], rhs=zrhs[:],
                             start=False, stop=False)
        for kt in range(KT):
            for mi_l in range(half_mi):
                msl = slice(mi_l * MCHUNK, (mi_l + 1) * MCHUNK)
                for dt_i in range(DT):
                    nc.tensor.matmul(
                        psums[mi_l][dt_i][:],
                        lhsT=weffs[kt][:, dt_i * 128:(dt_i + 1) * 128],
                        rhs=toks[kt][:, msl],
                        start=(kt == 0), stop=(kt == KT - 1))
            if kt < KT - 1:
                for _ in range(n_dummy):
                    nc.tensor.matmul(psums[0][0][:], lhsT=zmm[:], rhs=zrhs[:],
                                     start=False, stop=False)
        for mi_l in range(half_mi):
            msl = slice(mi_l * MCHUNK, (mi_l + 1) * MCHUNK)
            for dt_i in range(DT):
                evict(psums[mi_l][dt_i], dt_i, msl)

        # half 1: everything is SBUF-resident by now, so go psum-outer --
        # each output group evicts right after its 8 matmuls, overlapping
        # the remaining groups' matmuls instead of trailing them all
        for mi_l in range(half_mi):
            mi = half_mi + mi_l
            msl = slice(mi * MCHUNK, (mi + 1) * MCHUNK)
            for dt_i in range(DT):
                ps = psum_pool.tile([128, MCHUNK], f32, name="ps", tag="ps")
                for kt in range(KT):
                    nc.tensor.matmul(
                        ps[:],
                        lhsT=weffs[kt][:, dt_i * 128:(dt_i + 1) * 128],
                        rhs=toks[kt][:, msl],
                        start=(kt == 0), stop=(kt == KT - 1))
                evict(ps, dt_i, msl)

    nc.compile()
    return nc


def _get_nc(mode: str) -> bass.Bass:
    if mode not in _BUILD_CACHE:
        _BUILD_CACHE[mode] = _build(mode)
    return _BUILD_CACHE[mode]


def _make_in_maps(boundary_tokens, W_b2b, b_b2b, mode):
    wire = np.float16 if mode == "f16" else np.float32
    tok = np.ascontiguousarray(
        np.asarray(boundary_tokens, dtype=np.float32)
        .reshape(BN, D_MODEL).T.astype(wire))
    W = np.asarray(W_b2b, dtype=np.float32).astype(wire).reshape(
        D_MODEL, BULK_DIM, D_MODEL)
    b = np.asarray(b_b2b, dtype=np.float32).reshape(BULK_DIM, D_MODEL)
    in_maps = []
    for c in range(NCORES):
        f, t = divmod(c, T_SHARDS)
        dsl = slice(f * DS, (f + 1) * DS)
        in_maps.append({
            "tokT": np.ascontiguousarray(tok[:, t * MS:(t + 1) * MS]),
            "wsl": np.ascontiguousarray(
                W[:, :, dsl].reshape(D_MODEL, BULK_DIM * DS)),
            "bsl": np.ascontiguousarray(b[:, dsl].T),
        })
    return in_maps


def _assemble(results):
    out = np.empty((BN, D_MODEL), dtype=np.float32)
    for c in range(NCORES):
        f, t = divmod(c, T_SHARDS)
        out[t * MS:(t + 1) * MS, f * DS:(f + 1) * DS] = results[c]["outT"].T
    return out.reshape(B, N, D_MODEL)


def run(boundary_tokens, W_b2b, b_b2b, mode=None, **spmd_kwargs):
    mode = mode or MODE
    nc = _get_nc(mode)
    in_maps = _make_in_maps(boundary_tokens, W_b2b, b_b2b, mode)
    res = run_bass_kernel_spmd(nc, in_maps, list(range(NCORES)), **spmd_kwargs)
    return _assemble(res.results), res


def kernel(boundary_tokens, W_b2b, b_b2b):
    out, _ = run(boundary_tokens, W_b2b, b_b2b)
    return out

